# revision 8
# baseline (speedup 1.0000x reference)
"""Trainium2 Bass kernel for nn_Attentional_Aggregation (segment softmax attention).

Math (reference):
    keys_i = emb_i @ Wk.T + bk
    q_g    = emb[last(g)] @ Wq.T + bq
    logit_i = <q_{g(i)}, keys_i>
    w = segment_softmax(logit)
    out_g = sum_{i in g} w_i * keys_i

Reformulation (softmax weights sum to 1 per group, so keys are never
materialized per element):
    logit_i = <qk_{g(i)}, emb_i> + cq_{g(i)}
        qk_g = embL_g @ (Wq.T @ Wk) + bq @ Wk      (device phase A)
        cq_g = embL_g @ (Wq.T @ bk) + bq @ bk      (device phase A, col 128)
    e_i   = exp(logit_i)           (no max-subtraction: |logit| < ~21, fp32/bf16 safe)
    out_g = (sum e_i emb_i / sum e_i) @ Wk.T + bk  (device: Wk @ S and denom;
                                                    host: divide + bias)

Sharding: groups (and their contiguous element ranges) are partitioned across
8 cores; each core processes 98 blocks of 128 groups. Elements of each block
are padded to PB*128 so the program is fully static (SPMD: one program,
per-core data). Per-element qk rows are fetched with indirect DMA (gather)
from a per-core DRAM table; the segment scatter is a one-hot matmul into PSUM.
"""

import os
import numpy as np
import ml_dtypes

import concourse.bacc as bacc
import concourse.bass as bass
import concourse.mybir as mybir
import concourse.tile as tile
from concourse.bass_utils import run_bass_kernel_spmd

BF16 = ml_dtypes.bfloat16

N = 1_000_000
G = 100_000
D = 128
NCORES = 8
NBLK_FULL = 98
GC_FULL = NBLK_FULL * 128   # 12544 groups per core; last core padded

# Exposed for test harness
LAST_EXEC_NS = None
LAST_RESULTS = None

_cache = {}


def _build_program(PB, NBLK=NBLK_FULL, NCORES=NCORES, enable_asserts=False):
    """Build the SPMD Bass program (same for all cores)."""
    GC = NBLK * 128
    f32 = mybir.dt.float32
    bf16 = mybir.dt.bfloat16
    i32 = mybir.dt.int32
    ts = bass.ts

    nc = bacc.Bacc(
        "TRN2",
        target_bir_lowering=False,
        debug=False,
        enable_asserts=enable_asserts,
        num_devices=NCORES,
    )

    # Inputs (per-core data)
    embp = nc.dram_tensor("embp", [NBLK, 128, PB * 129], bf16, kind="ExternalInput").ap()
    segrel = nc.dram_tensor("segrel", [NBLK, 128, PB], f32, kind="ExternalInput").ap()
    segloc = nc.dram_tensor("segloc", [NBLK, 128, PB], i32, kind="ExternalInput").ap()
    embLT = nc.dram_tensor("embLT", [128, GC], bf16, kind="ExternalInput").ap()
    # Constants (identical across cores)
    armx = nc.dram_tensor("armx", [128, 129], bf16, kind="ExternalInput").ap()
    urow = nc.dram_tensor("urow", [1, 129], bf16, kind="ExternalInput").ap()
    wkt = nc.dram_tensor("wkt", [128, 128], bf16, kind="ExternalInput").ap()
    iota = nc.dram_tensor("iota", [128, 128], bf16, kind="ExternalInput").ap()
    onesr = nc.dram_tensor("onesr", [1, 128], bf16, kind="ExternalInput").ap()
    # Outputs
    outT = nc.dram_tensor("outT", [128, GC], f32, kind="ExternalOutput").ap()
    dens = nc.dram_tensor("dens", [1, GC], f32, kind="ExternalOutput").ap()
    # Internal scratch: per-group [qk | cq] rows
    qks = nc.dram_tensor("qks", [GC, 129], bf16).ap()

    with tile.TileContext(nc) as tc:
        with (
            tc.tile_pool(name="cpool", bufs=1) as cpool,
            tc.tile_pool(name="apool", bufs=3) as apool,
            tc.tile_pool(name="apsum", bufs=2, space="PSUM") as apsum,
            tc.tile_pool(name="aout", bufs=3) as aout,
            tc.tile_pool(name="bemb", bufs=3) as bemb,
            tc.tile_pool(name="bmeta", bufs=3) as bmeta,
            tc.tile_pool(name="bqkx", bufs=3) as bqkx,
            tc.tile_pool(name="bjunk", bufs=2) as bjunk,
            tc.tile_pool(name="blog", bufs=3) as blog,
            tc.tile_pool(name="boh", bufs=3) as boh,
            tc.tile_pool(name="bps", bufs=2, space="PSUM") as bps,
            tc.tile_pool(name="bpd", bufs=2, space="PSUM") as bpd,
            tc.tile_pool(name="bpo", bufs=2, space="PSUM") as bpo,
            tc.tile_pool(name="bsb", bufs=2) as bsb,
        ):
            # ---- load constants ----
            armx_sb = cpool.tile([128, 129], bf16)
            nc.sync.dma_start(out=armx_sb[:], in_=armx)
            urow_sb = cpool.tile([1, 129], bf16)
            nc.sync.dma_start(out=urow_sb[:], in_=urow)
            wkt_sb = cpool.tile([128, 128], bf16)
            nc.sync.dma_start(out=wkt_sb[:], in_=wkt)
            iota_sb = cpool.tile([128, 128], bf16)
            nc.sync.dma_start(out=iota_sb[:], in_=iota)
            onesr_sb = cpool.tile([1, 128], bf16)
            nc.sync.dma_start(out=onesr_sb[:], in_=onesr)
            den_all = cpool.tile([1, GC], f32)

            # ---- phase A: [qk | cq] rows for all groups ----
            for t in range(NBLK):
                lt = apool.tile([128, 128], bf16)
                nc.sync.dma_start(out=lt[:], in_=embLT[:, ts(t, 128)])
                qp = apsum.tile([128, 129], f32, space="PSUM")
                nc.tensor.matmul(qp[:], lhsT=lt[:], rhs=armx_sb[:], start=True, stop=False)
                nc.tensor.matmul(qp[:], lhsT=onesr_sb[:], rhs=urow_sb[:], start=False, stop=True)
                qs = aout.tile([128, 129], bf16)
                nc.scalar.copy(qs[:], qp[:])
                nc.sync.dma_start(out=qks[ts(t, 128), :], in_=qs[:])

            # ---- phase B: stream elements, segment softmax + scatter ----
            for b in range(NBLK):
                embt = bemb.tile([128, PB * 129], bf16)
                nc.sync.dma_start(out=embt[:], in_=embp[b])
                srel = bmeta.tile([128, PB], f32)
                nc.sync.dma_start(out=srel[:], in_=segrel[b])
                sloc = bmeta.tile([128, PB], i32)
                nc.sync.dma_start(out=sloc[:], in_=segloc[b])

                qkx = bqkx.tile([128, PB * 129], bf16)
                logits = blog.tile([128, PB], f32)
                for t in range(PB):
                    nc.gpsimd.indirect_dma_start(
                        out=qkx[:, t * 129 : (t + 1) * 129],
                        out_offset=None,
                        in_=qks,
                        in_offset=bass.IndirectOffsetOnAxis(ap=sloc[:, t : t + 1], axis=0),
                    )
                    junk = bjunk.tile([128, 129], bf16)
                    nc.vector.scalar_tensor_tensor(
                        out=junk[:],
                        in0=qkx[:, t * 129 : (t + 1) * 129],
                        scalar=1.0,
                        in1=embt[:, t * 129 : (t + 1) * 129],
                        op0=mybir.AluOpType.mult,
                        op1=mybir.AluOpType.mult,
                        accum_out=logits[:, t : t + 1],
                    )
                e_blk = blog.tile([128, PB], f32)
                nc.scalar.activation(e_blk[:], logits[:], mybir.ActivationFunctionType.Exp)

                stp = bps.tile([128, 128], f32, space="PSUM")
                dnp = bpd.tile([1, 128], f32, space="PSUM")
                for t in range(PB):
                    oh = boh.tile([128, 128], bf16)
                    nc.vector.tensor_scalar(
                        out=oh[:],
                        in0=iota_sb[:],
                        scalar1=srel[:, t : t + 1],
                        scalar2=e_blk[:, t : t + 1],
                        op0=mybir.AluOpType.is_equal,
                        op1=mybir.AluOpType.mult,
                    )
                    nc.tensor.matmul(
                        stp[:], lhsT=embt[:, t * 129 : t * 129 + 128], rhs=oh[:],
                        start=(t == 0), stop=(t == PB - 1),
                    )
                    nc.tensor.matmul(
                        dnp[:], lhsT=embt[:, t * 129 + 128 : (t + 1) * 129], rhs=oh[:],
                        start=(t == 0), stop=(t == PB - 1),
                    )
                sts = bsb.tile([128, 128], bf16)
                nc.scalar.copy(sts[:], stp[:])
                otp = bpo.tile([128, 128], f32, space="PSUM")
                nc.tensor.matmul(otp[:], lhsT=wkt_sb[:], rhs=sts[:], start=True, stop=True)
                ots = bsb.tile([128, 128], f32)
                nc.scalar.copy(ots[:], otp[:])
                nc.sync.dma_start(out=outT[:, ts(b, 128)], in_=ots[:])
                nc.scalar.copy(den_all[:, ts(b, 128)], dnp[:])

            nc.sync.dma_start(out=dens, in_=den_all[:])

    nc.compile()
    return nc


def _host_prep(embeddings, seg_ids, Wq, bq, Wk, bk, NBLK=NBLK_FULL, ncores=NCORES,
               num_groups=G):
    """Shard + pad inputs per core; compute host-side constants.

    Returns (PB, in_maps). Works for synthetic small configs too.
    """
    GC = NBLK * 128
    emb = np.ascontiguousarray(embeddings, dtype=np.float32)
    seg = np.ascontiguousarray(seg_ids, dtype=np.int64)

    counts = np.bincount(seg, minlength=num_groups)
    last_idx = np.cumsum(counts) - 1

    ARm = (Wq.T @ Wk).astype(np.float32)
    uvec = (bq @ Wk).astype(np.float32)
    vvec = (Wq.T @ bk).astype(np.float32)
    c0 = float(bq @ bk)
    armx = np.concatenate([ARm, vvec[:, None]], axis=1)      # [128, 129]
    urowx = np.concatenate([uvec, [c0]]).reshape(1, 129)     # [1, 129]

    # PB: uniform tile count per 128-group block across all cores
    PB = 0
    core_meta = []
    for c in range(ncores):
        g0 = c * GC
        g1 = min((c + 1) * GC, num_groups)
        e0 = int(np.searchsorted(seg, g0, "left"))
        e1 = int(np.searchsorted(seg, g1, "left"))
        if g0 >= num_groups:
            core_meta.append((g0, g0, e0, e0))
            continue
        blen = np.bincount((seg[e0:e1] - g0) // 128, minlength=NBLK)
        PB = max(PB, int(np.ceil(blen.max() / 128)))
        core_meta.append((g0, g1, e0, e1))

    emb_bf = emb.astype(BF16)

    in_maps = []
    iota = np.tile(np.arange(128, dtype=np.float32), (128, 1)).astype(BF16)
    consts = dict(
        armx=armx.astype(BF16),
        urow=urowx.astype(BF16),
        wkt=np.ascontiguousarray(Wk.T.astype(np.float32)).astype(BF16),
        iota=iota,
        onesr=np.ones((1, 128), dtype=BF16),
    )
    for c in range(ncores):
        g0, g1, e0, e1 = core_meta[c]
        ne = e1 - e0
        segc = seg[e0:e1] - g0              # local group ids [0, GC)
        blk = segc // 128                   # block of each element
        bstart = np.searchsorted(segc, np.arange(NBLK) * 128, "left")
        pos = np.arange(ne, dtype=np.int64) - bstart[blk]
        tt = pos // 128                     # tile slot within block
        pp = pos % 128                      # partition within tile

        # element-major padded arrays -> partition-major device layout
        embp = np.zeros((NBLK, 128, PB, 129), dtype=BF16)   # [b, p, t, c]
        embp[:, :, :, 128] = BF16(1.0)
        embp[blk, pp, tt, :128] = emb_bf[e0:e1]
        embp = embp.reshape(NBLK, 128, PB * 129)

        segrel = np.full((NBLK, 128, PB), -1.0, dtype=np.float32)
        segrel[blk, pp, tt] = (segc - blk * 128).astype(np.float32)
        segloc = np.zeros((NBLK, 128, PB), dtype=np.int32)
        segloc[blk, pp, tt] = segc.astype(np.int32)

        embLT = np.zeros((128, GC), dtype=BF16)
        embLT[:, : g1 - g0] = emb_bf[last_idx[g0:g1]].T

        m = dict(
            embp=np.ascontiguousarray(embp),
            segrel=np.ascontiguousarray(segrel),
            segloc=np.ascontiguousarray(segloc),
            embLT=np.ascontiguousarray(embLT),
        )
        m.update(consts)
        in_maps.append(m)
    return PB, in_maps


def kernel(embeddings, seg_ids, Wq, bq, Wk, bk):
    global LAST_EXEC_NS, LAST_RESULTS
    Wq = np.asarray(Wq, dtype=np.float32)
    bq = np.asarray(bq, dtype=np.float32)
    Wk = np.asarray(Wk, dtype=np.float32)
    bk = np.asarray(bk, dtype=np.float32)
    embeddings = np.asarray(embeddings)
    seg_ids = np.asarray(seg_ids)

    PB, in_maps = _host_prep(embeddings, seg_ids, Wq, bq, Wk, bk)

    if PB not in _cache:
        _cache[PB] = _build_program(PB)
    nc = _cache[PB]

    trace = bool(int(os.environ.get("BASS_KERNEL_TRACE", "0")))
    res = run_bass_kernel_spmd(nc, in_maps, core_ids=list(range(NCORES)), trace=trace)
    LAST_RESULTS = res
    LAST_EXEC_NS = res.exec_time_ns

    out = np.empty((G, D), dtype=np.float32)
    for c in range(NCORES):
        g0 = c * GC_FULL
        g1 = min((c + 1) * GC_FULL, G)
        oT = res.results[c]["outT"][:, : g1 - g0]
        dn = res.results[c]["dens"][0, : g1 - g0]
        out[g0:g1] = oT.T / dn[:, None] + bk
    return out


# revision 17
# speedup vs baseline: 1.8946x; 1.8946x over previous
"""Trainium2 Bass kernel for nn_Attentional_Aggregation (segment softmax attention).

Math (reference):
    keys_i = emb_i @ Wk.T + bk
    q_g    = emb[last(g)] @ Wq.T + bq
    logit_i = <q_{g(i)}, keys_i>
    w = segment_softmax(logit)
    out_g = sum_{i in g} w_i * keys_i

Reformulation:
    logit_i = <qk_{g(i)}, emb_i> + cq_{g(i)},  qk_g = embL_g @ (Wq.T Wk) + bq Wk
    The additive cq term scales numerator and denominator of the softmax
    identically, so it cancels and is never computed.
    out_g = (sum e_i emb_i / sum e_i) @ Wk.T + bk   (device: Wk @ S and denom;
                                                     host: divide + bias)

Device strategy (per core, fully static SPMD program):
  phase A: qkT[c, g] = ARm.T @ embLT  (+ u bias via ACT), kept SBUF-resident.
  phase B per 128-group block (PB element tiles of 128):
    - natural emb tile   [i, 129]  (bf16, col 128 = 1) for the scatter
    - transposed emb tile[c, i]    (fp16, via DMA-transpose) for the pairs matmul
    - L[i, g]  = embT_t.T @ qkT_blk          (PE, all-pairs logits)
    - E[i, g]  = exp(L)                       (ACT)
    - ME[i, g] = (iota == segrel_i) * E       (DVE, fused one-hot mask)
    - S[c, g] += emb_t.T @ ME ; den[1,g] += ones.T @ ME   (PE, PSUM accum)
    - outT = Wk @ S (PE); copies (DVE); host divides by den and adds bk.

Sharding: 8 cores x 12544 groups (98 blocks); elements of each block padded
to PB*128 (PB derived from the data, typically 11-12).
"""

import os
import numpy as np
import ml_dtypes

import concourse.bacc as bacc
import concourse.bass as bass
import concourse.mybir as mybir
import concourse.tile as tile
from concourse.bass_utils import run_bass_kernel_spmd

BF16 = ml_dtypes.bfloat16
FP16 = np.float16

N = 1_000_000
G = 100_000
D = 128
NCORES = 8
NBLK_FULL = 98
GC_FULL = NBLK_FULL * 128   # 12544 groups per core; last core padded

# Exposed for test harness
LAST_EXEC_NS = None
LAST_RESULTS = None

_cache = {}


def _build_program(PB, NBLK=NBLK_FULL, NCORES=NCORES, enable_asserts=False):
    """Build the SPMD Bass program (same for all cores)."""
    GC = NBLK * 128
    f32 = mybir.dt.float32
    bf16 = mybir.dt.bfloat16
    f16 = mybir.dt.float16
    ts = bass.ts

    nc = bacc.Bacc(
        "TRN2",
        target_bir_lowering=False,
        debug=False,
        enable_asserts=enable_asserts,
        num_devices=NCORES,
    )

    # Inputs (per-core data)
    embp = nc.dram_tensor("embp", [NBLK, 128, PB * 129], bf16, kind="ExternalInput").ap()
    embtt = nc.dram_tensor("embtt", [NBLK * PB * 128, 128], f16, kind="ExternalInput").ap()
    segrel = nc.dram_tensor("segrel", [NBLK, 128, PB], f32, kind="ExternalInput").ap()
    embLT = nc.dram_tensor("embLT", [128, GC], f16, kind="ExternalInput").ap()
    # Constants (identical across cores)
    arm = nc.dram_tensor("arm", [128, 128], f16, kind="ExternalInput").ap()
    ucol = nc.dram_tensor("ucol", [128, 1], f32, kind="ExternalInput").ap()
    wkt = nc.dram_tensor("wkt", [128, 128], bf16, kind="ExternalInput").ap()
    iota = nc.dram_tensor("iota", [128, 128], bf16, kind="ExternalInput").ap()
    # Outputs
    outT = nc.dram_tensor("outT", [128, GC], f32, kind="ExternalOutput").ap()
    dens = nc.dram_tensor("dens", [1, GC], f32, kind="ExternalOutput").ap()

    with tile.TileContext(nc) as tc:
        with (
            tc.tile_pool(name="cpool", bufs=1) as cpool,
            tc.tile_pool(name="apool", bufs=3) as apool,
            tc.tile_pool(name="apsum", bufs=1, space="PSUM") as apsum,
            tc.tile_pool(name="bemb", bufs=3) as bemb,
            tc.tile_pool(name="bembt", bufs=3) as bembt,
            tc.tile_pool(name="bmeta", bufs=3) as bmeta,
            tc.tile_pool(name="bpl", bufs=2, space="PSUM") as bpl,
            tc.tile_pool(name="be", bufs=4) as be,
            tc.tile_pool(name="bme", bufs=4) as bme,
            tc.tile_pool(name="bps", bufs=2, space="PSUM") as bps,
            tc.tile_pool(name="bpd", bufs=2, space="PSUM") as bpd,
            tc.tile_pool(name="bpo", bufs=1, space="PSUM") as bpo,
            tc.tile_pool(name="bsb", bufs=2) as bsb,
        ):
            # ---- constants ----
            arm_sb = cpool.tile([128, 128], f16)
            nc.sync.dma_start(out=arm_sb[:], in_=arm)
            ucol_sb = cpool.tile([128, 1], f32)
            nc.sync.dma_start(out=ucol_sb[:], in_=ucol)
            wkt_sb = cpool.tile([128, 128], bf16)
            nc.sync.dma_start(out=wkt_sb[:], in_=wkt)
            iota_sb = cpool.tile([128, 128], bf16)
            nc.sync.dma_start(out=iota_sb[:], in_=iota)
            den_all = cpool.tile([1, GC], f32)
            qkT = cpool.tile([128, GC], f16)      # SBUF-resident qk table

            # ---- phase A: qkT[c, g] ----
            for t in range(NBLK):
                lt = apool.tile([128, 128], f16)
                nc.sync.dma_start(out=lt[:], in_=embLT[:, ts(t, 128)])
                qp = apsum.tile([128, 128], f32, space="PSUM")
                nc.tensor.matmul(qp[:], lhsT=arm_sb[:], rhs=lt[:], start=True, stop=True)
                nc.scalar.activation(
                    qkT[:, ts(t, 128)], qp[:],
                    mybir.ActivationFunctionType.Identity, bias=ucol_sb[:],
                )

            # ---- phase B ----
            for b in range(NBLK):
                embt = bemb.tile([128, PB * 129], bf16)
                nc.sync.dma_start(out=embt[:], in_=embp[b])
                embT = bembt.tile([128, PB * 128], f16)
                nc.sync.dma_start_transpose(
                    out=embT[:], in_=embtt[b * PB * 128 : (b + 1) * PB * 128, :]
                )
                srel = bmeta.tile([128, PB], f32)
                nc.sync.dma_start(out=srel[:], in_=segrel[b])

                stp = bps.tile([128, 128], f32, space="PSUM")
                dnp = bpd.tile([1, 128], f32, space="PSUM")
                for t in range(PB):
                    lp = bpl.tile([128, 128], f32, space="PSUM")
                    nc.tensor.matmul(
                        lp[:], lhsT=embT[:, ts(t, 128)], rhs=qkT[:, ts(b, 128)],
                        start=True, stop=True,
                    )
                    et = be.tile([128, 128], bf16)
                    nc.scalar.activation(et[:], lp[:], mybir.ActivationFunctionType.Exp)
                    me = bme.tile([128, 128], bf16)
                    nc.vector.scalar_tensor_tensor(
                        out=me[:], in0=iota_sb[:], scalar=srel[:, t : t + 1],
                        in1=et[:], op0=mybir.AluOpType.is_equal,
                        op1=mybir.AluOpType.mult,
                    )
                    nc.tensor.matmul(
                        stp[:], lhsT=embt[:, t * 129 : t * 129 + 128], rhs=me[:],
                        start=(t == 0), stop=(t == PB - 1),
                    )
                    nc.tensor.matmul(
                        dnp[:], lhsT=embt[:, t * 129 + 128 : (t + 1) * 129], rhs=me[:],
                        start=(t == 0), stop=(t == PB - 1),
                    )
                sts = bsb.tile([128, 128], bf16)
                nc.vector.tensor_copy(sts[:], stp[:])
                otp = bpo.tile([128, 128], f32, space="PSUM")
                nc.tensor.matmul(otp[:], lhsT=wkt_sb[:], rhs=sts[:], start=True, stop=True)
                ots = bsb.tile([128, 128], f32)
                nc.vector.tensor_copy(ots[:], otp[:])
                nc.sync.dma_start(out=outT[:, ts(b, 128)], in_=ots[:])
                nc.vector.tensor_copy(den_all[:, ts(b, 128)], dnp[:])

            nc.sync.dma_start(out=dens, in_=den_all[:])

    nc.compile()
    return nc


def _host_prep(embeddings, seg_ids, Wq, bq, Wk, bk, NBLK=NBLK_FULL, ncores=NCORES,
               num_groups=G):
    """Shard + pad inputs per core; compute host-side constants.

    Returns (PB, in_maps). Works for synthetic small configs too.
    """
    GC = NBLK * 128
    emb = np.ascontiguousarray(embeddings, dtype=np.float32)
    seg = np.ascontiguousarray(seg_ids, dtype=np.int64)

    counts = np.bincount(seg, minlength=num_groups)
    last_idx = np.cumsum(counts) - 1

    ARm = (Wq.T @ Wk).astype(np.float32)
    uvec = (bq @ Wk).astype(np.float32)

    # PB: uniform tile count per 128-group block across all cores
    PB = 0
    core_meta = []
    for c in range(ncores):
        g0 = c * GC
        g1 = min((c + 1) * GC, num_groups)
        e0 = int(np.searchsorted(seg, g0, "left"))
        e1 = int(np.searchsorted(seg, g1, "left"))
        if g0 >= num_groups:
            core_meta.append((g0, g0, e0, e0))
            continue
        blen = np.bincount((seg[e0:e1] - g0) // 128, minlength=NBLK)
        PB = max(PB, int(np.ceil(blen.max() / 128)))
        core_meta.append((g0, g1, e0, e1))

    emb_bf = emb.astype(BF16)
    emb_f16 = emb.astype(FP16)

    in_maps = []
    iota = np.tile(np.arange(128, dtype=np.float32), (128, 1)).astype(BF16)
    consts = dict(
        arm=ARm.astype(FP16),
        ucol=uvec.reshape(128, 1).astype(np.float32),
        wkt=np.ascontiguousarray(Wk.T.astype(np.float32)).astype(BF16),
        iota=iota,
    )
    for c in range(ncores):
        g0, g1, e0, e1 = core_meta[c]
        ne = e1 - e0
        segc = seg[e0:e1] - g0              # local group ids [0, GC)
        blk = segc // 128                   # block of each element
        bstart = np.searchsorted(segc, np.arange(NBLK) * 128, "left")
        pos = np.arange(ne, dtype=np.int64) - bstart[blk]
        tt = pos // 128                     # tile slot within block
        pp = pos % 128                      # partition within tile

        # natural layout (partition-major within block), ones in col 128
        embp = np.zeros((NBLK, 128, PB, 129), dtype=BF16)   # [b, p, t, c]
        embp[:, :, :, 128] = BF16(1.0)
        embp[blk, pp, tt, :128] = emb_bf[e0:e1]
        embp = embp.reshape(NBLK, 128, PB * 129)

        # element-major fp16 copy for the DMA-transpose load
        embtt = np.zeros((NBLK * PB * 128, 128), dtype=FP16)
        embtt[blk * PB * 128 + tt * 128 + pp] = emb_f16[e0:e1]

        segrel = np.full((NBLK, 128, PB), -1.0, dtype=np.float32)
        segrel[blk, pp, tt] = (segc - blk * 128).astype(np.float32)

        embLT = np.zeros((128, GC), dtype=FP16)
        embLT[:, : g1 - g0] = emb_f16[last_idx[g0:g1]].T

        m = dict(
            embp=np.ascontiguousarray(embp),
            embtt=embtt,
            segrel=np.ascontiguousarray(segrel),
            embLT=np.ascontiguousarray(embLT),
        )
        m.update(consts)
        in_maps.append(m)
    return PB, in_maps


def kernel(embeddings, seg_ids, Wq, bq, Wk, bk):
    global LAST_EXEC_NS, LAST_RESULTS
    Wq = np.asarray(Wq, dtype=np.float32)
    bq = np.asarray(bq, dtype=np.float32)
    Wk = np.asarray(Wk, dtype=np.float32)
    bk = np.asarray(bk, dtype=np.float32)
    embeddings = np.asarray(embeddings)
    seg_ids = np.asarray(seg_ids)

    PB, in_maps = _host_prep(embeddings, seg_ids, Wq, bq, Wk, bk)

    if PB not in _cache:
        _cache[PB] = _build_program(PB)
    nc = _cache[PB]

    trace = bool(int(os.environ.get("BASS_KERNEL_TRACE", "0")))
    res = run_bass_kernel_spmd(nc, in_maps, core_ids=list(range(NCORES)), trace=trace)
    LAST_RESULTS = res
    LAST_EXEC_NS = res.exec_time_ns

    out = np.empty((G, D), dtype=np.float32)
    for c in range(NCORES):
        g0 = c * GC_FULL
        g1 = min((c + 1) * GC_FULL, G)
        oT = res.results[c]["outT"][:, : g1 - g0]
        dn = res.results[c]["dens"][0, : g1 - g0]
        out[g0:g1] = oT.T / dn[:, None] + bk
    return out
